# revision 15
# baseline (speedup 1.0000x reference)
"""Low-rank orthogonal projection kernel for Trainium2 (8 NeuronCores).

Math: reference computes P = W @ W.T (W [D,r], orthonormal cols) and
    out = target @ (I-P).T + source @ P.T
P symmetric =>  out = target + (source - target) @ W @ W.T  (rank-r update).

Raw-Bass implementation (hand-rolled semaphores, one wait per instruction).
Pipelined over 4 "quarters" of 256 tokens per core with double buffering so
DMA streams continuously:

  per quarter q (256 tokens = 2 subtiles of 128):
    SP    : DMA in src/tgt quarter [128, 2, 4096] fp32 (double buffered)
    DVE   : diff = src - tgt  -> bf16 (one instr per quarter); runs one
            quarter AHEAD of the adds so PE never stalls on it
    PE    : transposes diff chunks into PSUM bf16 [128, 4, 256] (2 bank ring)
    ACT   : copy diffT PSUM->SBUF bf16 (one instr per 4 chunks)
    PE    : stage A   tA[64, 256] += W_chunk.T @ diffT_chunk   (bf16, fp32 acc)
    ACT   : copy tA PSUM->SBUF bf16 tT
    PE    : stage B   corr[128, 512] = tT_s.T @ WT_chunk       (bf16, fp32 acc)
    DVE   : out_tile = corr + tgt  (written into the dead src buffer)
    Q7    : DMA out quarter from src buffer (SWDGE queue so the load queue
            on SP never blocks on compute)
W and W.T are cast to bf16 on host (tiny) and passed as extra inputs.
"""

from contextlib import ExitStack

import numpy as np
from ml_dtypes import bfloat16

import concourse.bass as bass
import concourse.mybir as mybir
from concourse.bass_utils import run_bass_kernel_spmd

N_TOKENS = 8192
D = 4096
R = 64
N_CORES = 8
TOK_PER_CORE = N_TOKENS // N_CORES  # 1024
NQ = 4  # quarters per core
QT = TOK_PER_CORE // NQ  # 256 tokens per quarter
S = QT // 128  # 2 subtiles per quarter
DC = D // 128  # 32 contraction chunks
NB = D // 512  # 8 output column chunks
G = DC // 4  # 8 groups of 4 chunks (dT copy granularity)

F32 = mybir.dt.float32
BF16 = mybir.dt.bfloat16


def build_bass() -> bass.Bass:
    nc = bass.Bass()
    src = nc.declare_dram_parameter("source", [TOK_PER_CORE, D], F32, isOutput=False)
    tgt = nc.declare_dram_parameter("target", [TOK_PER_CORE, D], F32, isOutput=False)
    # weight is pre-rearranged on host to [128, DC, R] (w_r[p, o, r] =
    # W[o*128 + p, r]) so the load is one 4 KB contiguous chunk per partition
    w = nc.declare_dram_parameter("weight", [128, DC, R], BF16, isOutput=False)
    wt = nc.declare_dram_parameter("weight_t", [R, D], BF16, isOutput=False)
    out = nc.declare_dram_parameter("out", [TOK_PER_CORE, D], F32, isOutput=True)

    ctx = ExitStack()
    ident_f = ctx.enter_context(nc.sbuf_tensor("ident_f", [128, 128], F32))
    ident = ctx.enter_context(nc.sbuf_tensor("ident", [128, 128], BF16))
    w_sb = ctx.enter_context(nc.sbuf_tensor("w_sb", [128, DC, R], BF16))
    wt_sb = ctx.enter_context(nc.sbuf_tensor("wt_sb", [R, D], BF16))
    src_b = [
        ctx.enter_context(nc.sbuf_tensor(f"src{i}", [128, S, D], F32)) for i in range(2)
    ]
    tgt_b = [
        ctx.enter_context(nc.sbuf_tensor(f"tgt{i}", [128, S, D], F32)) for i in range(2)
    ]
    diff_b = [
        ctx.enter_context(nc.sbuf_tensor(f"diff{i}", [128, S, D], BF16))
        for i in range(2)
    ]
    dT_sb = ctx.enter_context(nc.sbuf_tensor("dT", [128, 2, 4, QT], BF16))
    tT = ctx.enter_context(nc.sbuf_tensor("tT", [R, 2, QT], BF16))
    p_dT = [
        ctx.enter_context(nc.psum_tensor(f"pdT{i}", [128, 4, QT], BF16))
        for i in range(2)
    ]
    p_tA = [
        ctx.enter_context(nc.psum_tensor(f"ptA{i}", [R, QT], F32)) for i in range(2)
    ]
    p_B = [
        ctx.enter_context(nc.psum_tensor(f"pB{i}", [128, 1024], F32)) for i in range(2)
    ]

    # DMA-completion semaphores are parity-split (ls0/ls1, lt0/lt1) or
    # serialized by explicit waits (ldw, st) so that each semaphore has at
    # most one DMA in flight: concurrent DMAs complete in nondeterministic
    # order, so a shared cumulative counter could release a waiter before the
    # DMA it actually depends on has landed.
    with (
        nc.Block() as block,
        nc.semaphore("ldw") as ldw,  # weight loads (serialized)
        nc.semaphore("ls0") as ls0,  # src loads, even quarters
        nc.semaphore("ls1") as ls1,  # src loads, odd quarters
        nc.semaphore("lt0") as lt0,  # tgt loads, even quarters
        nc.semaphore("lt1") as lt1,  # tgt loads, odd quarters
        nc.semaphore("idn") as idn,  # identity built + cast
        nc.semaphore("dv") as dv,    # subs done (1 per quarter)
        nc.semaphore("ts_") as ts_,  # transpose dc-chunks done (32 per quarter)
        nc.semaphore("cp") as cp,    # diffT psum->sbuf copies (8 per quarter)
        nc.semaphore("am") as am,    # stage-A matmuls (32 per quarter)
        nc.semaphore("tc_") as tc_,  # tT copies (1 per quarter)
        nc.semaphore("bm") as bm,    # stage-B matmuls (16 per quarter)
        nc.semaphore("ad") as ad,    # adds (8 per quarter)
        nc.semaphore("st") as st,    # stores (16 per DMA, serialized)
    ):
        ls = [ls0, ls1]
        lt = [lt0, lt1]

        @block.sync
        def _(sp):
            sp.dma_start(w_sb[:], w[:, :, :]).then_inc(ldw, 16)
            sp.wait_ge(ldw, 16)
            sp.dma_start(wt_sb[:], wt[:, :]).then_inc(ldw, 16)
            for q in range(NQ):
                rows = slice(q * QT, (q + 1) * QT)
                if q >= 2:
                    # src buffer freed once store(q-2) read it out
                    sp.wait_ge(st, (q - 1) * 16)
                sp.dma_start(
                    src_b[q % 2][:],
                    src[rows, :].rearrange("(s p) d -> p s d", p=128),
                ).then_inc(ls[q % 2], 16)
                if q >= 2:
                    # tgt buffer freed once adds(q-2) consumed it
                    sp.wait_ge(ad, (q - 1) * G)
                sp.dma_start(
                    tgt_b[q % 2][:],
                    tgt[rows, :].rearrange("(s p) d -> p s d", p=128),
                ).then_inc(lt[q % 2], 16)

        @block.gpsimd
        def _(g):
            g.memset(ident_f[:], 0.0).then_inc(idn, 1)
            g.wait_ge(idn, 1)
            g.affine_select(
                out=ident_f[:],
                in_=ident_f[:],
                compare_op=mybir.AluOpType.not_equal,
                fill=1.0,
                base=0,
                pattern=[[-1, 128]],
                channel_multiplier=1,
            ).then_inc(idn, 1)
            for q in range(NQ):
                rows = slice(q * QT, (q + 1) * QT)
                g.wait_ge(ad, (q + 1) * G)
                if q >= 1:
                    # serialize increments of st (one store in flight per sem)
                    g.wait_ge(st, q * 16)
                g.dma_start(
                    out[rows, :].rearrange("(s p) d -> p s d", p=128),
                    src_b[q % 2][:],
                ).then_inc(st, 16)

        @block.vector
        def _(ve):
            def sub(q):
                ve.wait_ge(ls[q % 2], (q // 2 + 1) * 16)
                ve.wait_ge(lt[q % 2], (q // 2 + 1) * 16)
                if q >= 2:
                    ve.wait_ge(ts_, (q - 1) * DC)
                ve.tensor_sub(
                    out=diff_b[q % 2][:], in0=src_b[q % 2][:], in1=tgt_b[q % 2][:]
                ).then_inc(dv, 1)

            def adds(q):
                for j in range(G):
                    s, c0 = j // 4, (2 * j) % NB
                    ve.wait_ge(bm, q * 16 + 2 * j + 2)
                    sl = (slice(None), s, slice(c0 * 512, (c0 + 2) * 512))
                    ve.tensor_add(
                        out=src_b[q % 2][sl], in0=p_B[j % 2][:], in1=tgt_b[q % 2][sl]
                    ).then_inc(ad, 1)

            sub(0)
            sub(1)
            adds(0)
            sub(2)
            adds(1)
            sub(3)
            adds(2)
            adds(3)

        @block.scalar
        def _(act):
            act.wait_ge(idn, 2)
            act.copy(out=ident[:], in_=ident_f[:]).then_inc(idn, 1)
            for q in range(NQ):
                for j in range(G):
                    act.wait_ge(ts_, q * DC + 4 * j + 4)
                    if not (q == 0 and j < 2):
                        # slots freed once A-group j-2 consumed them
                        act.wait_ge(am, q * DC + 4 * j - 4)
                    act.copy(out=dT_sb[:, j % 2, :, :], in_=p_dT[j % 2][:]).then_inc(
                        cp, 1
                    )
                act.wait_ge(am, (q + 1) * DC)
                if q >= 2:
                    act.wait_ge(bm, (q - 1) * 16)
                act.copy(out=tT[:, q % 2, :], in_=p_tA[q % 2][:]).then_inc(tc_, 1)

        @block.tensor
        def _(pe):
            pe.wait_ge(idn, 3)
            pe.wait_ge(ldw, 32)

            def a_group(q, g):
                for i in range(4):
                    dcc = 4 * g + i
                    if i == 0:
                        pe.wait_ge(cp, q * G + g + 1)
                        if dcc == 0 and q >= 2:
                            pe.wait_ge(tc_, q - 1)
                    pe.matmul(
                        p_tA[q % 2][:],
                        lhsT=w_sb[:, dcc, :],
                        rhs=dT_sb[:, g % 2, i, :],
                        start=(dcc == 0),
                        stop=(dcc == DC - 1),
                    ).then_inc(am, 1)

            for q in range(NQ):
                for g in range(G):
                    for i in range(4):
                        dcc = 4 * g + i
                        if dcc == 0:
                            pe.wait_ge(dv, q + 1)
                        for s in range(S):
                            t = pe.transpose(
                                p_dT[g % 2][:, i, s * 128 : (s + 1) * 128],
                                diff_b[q % 2][:, s, dcc * 128 : (dcc + 1) * 128],
                                ident[:],
                            )
                            if s == S - 1:
                                t.then_inc(ts_, 1)
                    if g >= 1:
                        a_group(q, g - 1)
                a_group(q, G - 1)
                pe.wait_ge(tc_, q + 1)
                for k in range(16):
                    s, nb = k // NB, k % NB
                    if k % 2 == 0 and q * G + k // 2 - 1 > 0:
                        # p_B pair freed once the add two pairs back is done
                        # (crosses quarter boundaries for k < 4)
                        pe.wait_ge(ad, q * G + k // 2 - 1)
                    pe.matmul(
                        p_B[(k // 2) % 2][:, (k % 2) * 512 : (k % 2 + 1) * 512],
                        lhsT=tT[:, q % 2, s * 128 : (s + 1) * 128],
                        rhs=wt_sb[:, nb * 512 : (nb + 1) * 512],
                        start=True,
                        stop=True,
                    ).then_inc(bm, 1)

    ctx.close()
    return nc


_nc_cache = None


def _run(source, target, weight, trace=False, tmpdir=None):
    global _nc_cache
    source = np.ascontiguousarray(np.asarray(source, dtype=np.float32))
    target = np.ascontiguousarray(np.asarray(target, dtype=np.float32))
    weight = np.asarray(weight, dtype=np.float32)
    # [D, R] -> [128, DC, R] with w_r[p, o, r] = W[o*128 + p, r]
    w_bf = np.ascontiguousarray(
        weight.reshape(DC, 128, R).transpose(1, 0, 2).astype(bfloat16)
    )
    wt_bf = np.ascontiguousarray(weight.T.astype(bfloat16))
    if _nc_cache is None:
        _nc_cache = build_bass()
    nc = _nc_cache
    in_maps = []
    for c in range(N_CORES):
        rows = slice(c * TOK_PER_CORE, (c + 1) * TOK_PER_CORE)
        in_maps.append(
            {
                "source": source[rows],
                "target": target[rows],
                "weight": w_bf,
                "weight_t": wt_bf,
            }
        )
    res = run_bass_kernel_spmd(
        nc, in_maps, list(range(N_CORES)), trace=trace, tmpdir=tmpdir
    )
    full = np.concatenate([res.results[c]["out"] for c in range(N_CORES)], axis=0)
    return full, res


def kernel(source, target, weight):
    full, _ = _run(source, target, weight)
    return full


# revision 21
# speedup vs baseline: 1.0468x; 1.0468x over previous
"""Low-rank orthogonal projection kernel for Trainium2 (8 NeuronCores).

Math: reference computes P = W @ W.T (W [D,r], orthonormal cols) and
    out = target @ (I-P).T + source @ P.T
P symmetric =>  out = target + (source - target) @ W @ W.T  (rank-r update).

Raw-Bass implementation (hand-rolled semaphores, one wait per instruction).
Pipelined over 4 "quarters" of 256 tokens per core with double buffering so
DMA streams continuously:

  per quarter q (256 tokens = 2 subtiles of 128):
    SP    : DMA in src/tgt quarter [128, 2, 4096] fp32 (double buffered)
    DVE   : diff = src - tgt  -> bf16 (one instr per quarter); runs one
            quarter AHEAD of the adds so PE never stalls on it
    PE    : transposes diff chunks into PSUM bf16 [128, 4, 256] (2 bank ring)
    ACT   : copy diffT PSUM->SBUF bf16 (one instr per 4 chunks)
    PE    : stage A   tA[64, 256] += W_chunk.T @ diffT_chunk   (bf16, fp32 acc)
    ACT   : copy tA PSUM->SBUF bf16 tT
    PE    : stage B   corr[128, 512] = tT_s.T @ WT_chunk       (bf16, fp32 acc)
    DVE   : out_tile = corr + tgt  (written into the dead src buffer)
    Q7    : DMA out quarter from src buffer (SWDGE queue so the load queue
            on SP never blocks on compute)
W and W.T are cast to bf16 on host (tiny) and passed as extra inputs.
"""

from contextlib import ExitStack

import numpy as np
from ml_dtypes import bfloat16

import concourse.bass as bass
import concourse.mybir as mybir
from concourse.bass_utils import run_bass_kernel_spmd

N_TOKENS = 8192
D = 4096
R = 64
N_CORES = 8
TOK_PER_CORE = N_TOKENS // N_CORES  # 1024
NQ = 4  # quarters per core
QT = TOK_PER_CORE // NQ  # 256 tokens per quarter
S = QT // 128  # 2 subtiles per quarter
DC = D // 128  # 32 contraction chunks
NB = D // 512  # 8 output column chunks
G = DC // 4  # 8 groups of 4 chunks (dT copy granularity)

F32 = mybir.dt.float32
BF16 = mybir.dt.bfloat16


def build_bass() -> bass.Bass:
    nc = bass.Bass()
    src = nc.declare_dram_parameter("source", [TOK_PER_CORE, D], F32, isOutput=False)
    tgt = nc.declare_dram_parameter("target", [TOK_PER_CORE, D], F32, isOutput=False)
    # weight is pre-rearranged on host to [128, DC, R] (w_r[p, o, r] =
    # W[o*128 + p, r]) so the load is one 4 KB contiguous chunk per partition
    w = nc.declare_dram_parameter("weight", [128, DC, R], BF16, isOutput=False)
    wt = nc.declare_dram_parameter("weight_t", [R, D], BF16, isOutput=False)
    out = nc.declare_dram_parameter("out", [TOK_PER_CORE, D], F32, isOutput=True)

    ctx = ExitStack()
    ident_f = ctx.enter_context(nc.sbuf_tensor("ident_f", [128, 128], F32))
    ident = ctx.enter_context(nc.sbuf_tensor("ident", [128, 128], BF16))
    w_sb = ctx.enter_context(nc.sbuf_tensor("w_sb", [128, DC, R], BF16))
    wt_sb = ctx.enter_context(nc.sbuf_tensor("wt_sb", [R, D], BF16))
    # src is triple-buffered: adds(q) write the out tile into the (dead) src
    # buffer and the store DMAs it out, so src(q) reload only needs
    # store(q-3) complete — off the per-quarter critical chain.
    src_b = [
        ctx.enter_context(nc.sbuf_tensor(f"src{i}", [128, S, D], F32)) for i in range(3)
    ]
    tgt_b = [
        ctx.enter_context(nc.sbuf_tensor(f"tgt{i}", [128, S, D], F32)) for i in range(2)
    ]
    # single diff buffer: sub(q) waits for the transposes of q-1 to finish
    diff = ctx.enter_context(nc.sbuf_tensor("diff", [128, S, D], BF16))
    dT_sb = ctx.enter_context(nc.sbuf_tensor("dT", [128, 2, 4, QT], BF16))
    tT = ctx.enter_context(nc.sbuf_tensor("tT", [R, 2, QT], BF16))
    p_dT = [
        ctx.enter_context(nc.psum_tensor(f"pdT{i}", [128, 4, QT], BF16))
        for i in range(2)
    ]
    p_tA = [
        ctx.enter_context(nc.psum_tensor(f"ptA{i}", [R, QT], F32)) for i in range(2)
    ]
    p_B = [
        ctx.enter_context(nc.psum_tensor(f"pB{i}", [128, 1024], F32)) for i in range(2)
    ]

    # DMA-completion semaphores are parity-split (ls0/ls1, lt0/lt1) or
    # serialized by explicit waits (ldw, st) so that each semaphore has at
    # most one DMA in flight: concurrent DMAs complete in nondeterministic
    # order, so a shared cumulative counter could release a waiter before the
    # DMA it actually depends on has landed.
    with (
        nc.Block() as block,
        nc.semaphore("ldw") as ldw,  # weight loads (serialized)
        nc.semaphore("ls0") as ls0,  # src loads, q%3 == 0
        nc.semaphore("ls1") as ls1,  # src loads, q%3 == 1
        nc.semaphore("ls2") as ls2,  # src loads, q%3 == 2
        nc.semaphore("lt0") as lt0,  # tgt loads, even quarters
        nc.semaphore("lt1") as lt1,  # tgt loads, odd quarters
        nc.semaphore("idn") as idn,  # identity built + cast
        nc.semaphore("dv") as dv,    # subs done (1 per quarter)
        nc.semaphore("ts_") as ts_,  # transpose dc-chunks done (32 per quarter)
        nc.semaphore("cp") as cp,    # diffT psum->sbuf copies (8 per quarter)
        nc.semaphore("am") as am,    # stage-A matmuls (32 per quarter)
        nc.semaphore("tc_") as tc_,  # tT copies (1 per quarter)
        nc.semaphore("bm") as bm,    # stage-B matmuls (16 per quarter)
        nc.semaphore("ad") as ad,    # adds (8 per quarter)
        nc.semaphore("st") as st,    # stores (16 per DMA, serialized)
    ):
        ls = [ls0, ls1, ls2]
        lt = [lt0, lt1]

        @block.sync
        def _(sp):
            sp.dma_start(w_sb[:], w[:, :, :]).then_inc(ldw, 16)
            sp.wait_ge(ldw, 16)
            sp.dma_start(wt_sb[:], wt[:, :]).then_inc(ldw, 16)
            for q in range(NQ):
                rows = slice(q * QT, (q + 1) * QT)
                # tgt first: its gate (adds of q-2) clears before src's gate
                # (store of q-3 fully DMA'd out)
                if q >= 2:
                    # tgt buffer freed once adds(q-2) consumed it
                    sp.wait_ge(ad, (q - 1) * G)
                sp.dma_start(
                    tgt_b[q % 2][:],
                    tgt[rows, :].rearrange("(s p) d -> p s d", p=128),
                ).then_inc(lt[q % 2], 16)
                if q >= 3:
                    # src buffer freed once store(q-3) read it out
                    sp.wait_ge(st, (q - 2) * 16)
                sp.dma_start(
                    src_b[q % 3][:],
                    src[rows, :].rearrange("(s p) d -> p s d", p=128),
                ).then_inc(ls[q % 3], 16)

        @block.gpsimd
        def _(g):
            g.memset(ident_f[:], 0.0).then_inc(idn, 1)
            g.wait_ge(idn, 1)
            g.affine_select(
                out=ident_f[:],
                in_=ident_f[:],
                compare_op=mybir.AluOpType.not_equal,
                fill=1.0,
                base=0,
                pattern=[[-1, 128]],
                channel_multiplier=1,
            ).then_inc(idn, 1)
            for q in range(NQ):
                rows = slice(q * QT, (q + 1) * QT)
                g.wait_ge(ad, (q + 1) * G)
                if q >= 1:
                    # serialize increments of st (one store in flight per sem)
                    g.wait_ge(st, q * 16)
                g.dma_start(
                    out[rows, :].rearrange("(s p) d -> p s d", p=128),
                    src_b[q % 3][:],
                ).then_inc(st, 16)

        @block.vector
        def _(ve):
            def sub(q):
                ve.wait_ge(ls[q % 3], (q // 3 + 1) * 16)
                ve.wait_ge(lt[q % 2], (q // 2 + 1) * 16)
                if q >= 1:
                    # single diff buffer: transposes of q-1 must be done
                    ve.wait_ge(ts_, q * DC)
                ve.tensor_sub(
                    out=diff[:], in0=src_b[q % 3][:], in1=tgt_b[q % 2][:]
                ).then_inc(dv, 1)

            def adds(q):
                for j in range(G):
                    s, c0 = j // 4, (2 * j) % NB
                    ve.wait_ge(bm, q * 16 + 2 * j + 2)
                    sl = (slice(None), s, slice(c0 * 512, (c0 + 2) * 512))
                    ve.tensor_add(
                        out=src_b[q % 3][sl], in0=p_B[j % 2][:], in1=tgt_b[q % 2][sl]
                    ).then_inc(ad, 1)

            sub(0)
            sub(1)
            adds(0)
            sub(2)
            adds(1)
            sub(3)
            adds(2)
            adds(3)

        @block.scalar
        def _(act):
            act.wait_ge(idn, 2)
            act.copy(out=ident[:], in_=ident_f[:]).then_inc(idn, 1)
            for q in range(NQ):
                for j in range(G):
                    act.wait_ge(ts_, q * DC + 4 * j + 4)
                    if not (q == 0 and j < 2):
                        # slots freed once A-group j-2 consumed them
                        act.wait_ge(am, q * DC + 4 * j - 4)
                    act.copy(out=dT_sb[:, j % 2, :, :], in_=p_dT[j % 2][:]).then_inc(
                        cp, 1
                    )
                act.wait_ge(am, (q + 1) * DC)
                if q >= 2:
                    act.wait_ge(bm, (q - 1) * 16)
                act.copy(out=tT[:, q % 2, :], in_=p_tA[q % 2][:]).then_inc(tc_, 1)

        @block.tensor
        def _(pe):
            pe.wait_ge(idn, 3)
            pe.wait_ge(ldw, 32)

            def a_group(q, g):
                for i in range(4):
                    dcc = 4 * g + i
                    if i == 0:
                        pe.wait_ge(cp, q * G + g + 1)
                        if dcc == 0 and q >= 2:
                            pe.wait_ge(tc_, q - 1)
                    pe.matmul(
                        p_tA[q % 2][:],
                        lhsT=w_sb[:, dcc, :],
                        rhs=dT_sb[:, g % 2, i, :],
                        start=(dcc == 0),
                        stop=(dcc == DC - 1),
                    ).then_inc(am, 1)

            for q in range(NQ):
                for g in range(G):
                    for i in range(4):
                        dcc = 4 * g + i
                        if dcc == 0:
                            pe.wait_ge(dv, q + 1)
                        for s in range(S):
                            t = pe.transpose(
                                p_dT[g % 2][:, i, s * 128 : (s + 1) * 128],
                                diff[:, s, dcc * 128 : (dcc + 1) * 128],
                                ident[:],
                            )
                            if s == S - 1:
                                t.then_inc(ts_, 1)
                    if g >= 1:
                        a_group(q, g - 1)
                a_group(q, G - 1)
                pe.wait_ge(tc_, q + 1)
                for k in range(16):
                    s, nb = k // NB, k % NB
                    if k % 2 == 0 and q * G + k // 2 - 1 > 0:
                        # p_B pair freed once the add two pairs back is done
                        # (crosses quarter boundaries for k < 4)
                        pe.wait_ge(ad, q * G + k // 2 - 1)
                    pe.matmul(
                        p_B[(k // 2) % 2][:, (k % 2) * 512 : (k % 2 + 1) * 512],
                        lhsT=tT[:, q % 2, s * 128 : (s + 1) * 128],
                        rhs=wt_sb[:, nb * 512 : (nb + 1) * 512],
                        start=True,
                        stop=True,
                    ).then_inc(bm, 1)

    ctx.close()
    return nc


_nc_cache = None


def _run(source, target, weight, trace=False, tmpdir=None):
    global _nc_cache
    source = np.ascontiguousarray(np.asarray(source, dtype=np.float32))
    target = np.ascontiguousarray(np.asarray(target, dtype=np.float32))
    weight = np.asarray(weight, dtype=np.float32)
    # [D, R] -> [128, DC, R] with w_r[p, o, r] = W[o*128 + p, r]
    w_bf = np.ascontiguousarray(
        weight.reshape(DC, 128, R).transpose(1, 0, 2).astype(bfloat16)
    )
    wt_bf = np.ascontiguousarray(weight.T.astype(bfloat16))
    if _nc_cache is None:
        _nc_cache = build_bass()
    nc = _nc_cache
    in_maps = []
    for c in range(N_CORES):
        rows = slice(c * TOK_PER_CORE, (c + 1) * TOK_PER_CORE)
        in_maps.append(
            {
                "source": source[rows],
                "target": target[rows],
                "weight": w_bf,
                "weight_t": wt_bf,
            }
        )
    res = run_bass_kernel_spmd(
        nc, in_maps, list(range(N_CORES)), trace=trace, tmpdir=tmpdir
    )
    full = np.concatenate([res.results[c]["out"] for c in range(N_CORES)], axis=0)
    return full, res


def kernel(source, target, weight):
    full, _ = _run(source, target, weight)
    return full


# revision 24
# speedup vs baseline: 1.2035x; 1.1497x over previous
"""Low-rank orthogonal projection kernel for Trainium2 (8 NeuronCores).

Math: reference computes P = W @ W.T (W [D,r], orthonormal cols) and
    out = target @ (I-P).T + source @ P.T
P symmetric =>  out = target + (source - target) @ W @ W.T  (rank-r update).

Raw-Bass implementation (hand-rolled semaphores, one wait per instruction).
Pipelined over 4 "quarters" of 256 tokens per core with double buffering so
DMA streams continuously:

  per quarter q (256 tokens = 2 subtiles of 128):
    SP    : DMA in src/tgt quarter [128, 2, 4096] fp32 (double buffered)
    DVE   : diff = src - tgt  -> bf16 (one instr per quarter); runs one
            quarter AHEAD of the adds so PE never stalls on it
    PE    : transposes diff chunks into PSUM bf16 [128, 4, 256] (2 bank ring)
    ACT   : copy diffT PSUM->SBUF bf16 (one instr per 4 chunks)
    PE    : stage A   tA[64, 256] += W_chunk.T @ diffT_chunk   (bf16, fp32 acc)
    ACT   : copy tA PSUM->SBUF bf16 tT
    PE    : stage B   corr[128, 512] = tT_s.T @ WT_chunk       (bf16, fp32 acc)
    DVE   : out_tile = corr + tgt  (written into the dead src buffer)
    Q7    : DMA out quarter from src buffer (SWDGE queue so the load queue
            on SP never blocks on compute)
W and W.T are cast to bf16 on host (tiny) and passed as extra inputs.
"""

from contextlib import ExitStack

import numpy as np
from ml_dtypes import bfloat16

import concourse.bass as bass
import concourse.mybir as mybir
from concourse.bass_utils import run_bass_kernel_spmd

N_TOKENS = 8192
D = 4096
R = 64
N_CORES = 8
TOK_PER_CORE = N_TOKENS // N_CORES  # 1024
NQ = 4  # quarters per core
QT = TOK_PER_CORE // NQ  # 256 tokens per quarter
S = QT // 128  # 2 subtiles per quarter
DC = D // 128  # 32 contraction chunks
NB = D // 512  # 8 output column chunks
G = DC // 4  # 8 groups of 4 chunks (dT copy granularity)

F32 = mybir.dt.float32
BF16 = mybir.dt.bfloat16


def build_bass() -> bass.Bass:
    nc = bass.Bass()
    src = nc.declare_dram_parameter("source", [TOK_PER_CORE, D], F32, isOutput=False)
    tgt = nc.declare_dram_parameter("target", [TOK_PER_CORE, D], F32, isOutput=False)
    # weight is pre-rearranged on host to [128, DC, R] (w_r[p, o, r] =
    # W[o*128 + p, r]) so the load is one 4 KB contiguous chunk per partition
    w = nc.declare_dram_parameter("weight", [128, DC, R], BF16, isOutput=False)
    wt = nc.declare_dram_parameter("weight_t", [R, D], BF16, isOutput=False)
    out = nc.declare_dram_parameter("out", [TOK_PER_CORE, D], F32, isOutput=True)

    ctx = ExitStack()
    ident_f = ctx.enter_context(nc.sbuf_tensor("ident_f", [128, 128], F32))
    ident = ctx.enter_context(nc.sbuf_tensor("ident", [128, 128], BF16))
    w_sb = ctx.enter_context(nc.sbuf_tensor("w_sb", [128, DC, R], BF16))
    wt_sb = ctx.enter_context(nc.sbuf_tensor("wt_sb", [R, D], BF16))
    # src is triple-buffered: adds(q) write the out tile into the (dead) src
    # buffer and the store DMAs it out, so src(q) reload only needs
    # store(q-3) complete — off the per-quarter critical chain.
    src_b = [
        ctx.enter_context(nc.sbuf_tensor(f"src{i}", [128, S, D], F32)) for i in range(3)
    ]
    tgt_b = [
        ctx.enter_context(nc.sbuf_tensor(f"tgt{i}", [128, S, D], F32)) for i in range(2)
    ]
    # single diff buffer: sub(q) waits for the transposes of q-1 to finish
    diff = ctx.enter_context(nc.sbuf_tensor("diff", [128, S, D], BF16))
    dT_sb = ctx.enter_context(nc.sbuf_tensor("dT", [128, 2, 4, QT], BF16))
    tT = ctx.enter_context(nc.sbuf_tensor("tT", [R, 2, QT], BF16))
    p_dT = [
        ctx.enter_context(nc.psum_tensor(f"pdT{i}", [128, 4, QT], BF16))
        for i in range(2)
    ]
    p_tA = [
        ctx.enter_context(nc.psum_tensor(f"ptA{i}", [R, QT], F32)) for i in range(2)
    ]
    p_B = [
        ctx.enter_context(nc.psum_tensor(f"pB{i}", [128, 1024], F32)) for i in range(2)
    ]

    # DMA-completion semaphores are parity-split (ls0/ls1, lt0/lt1) or
    # serialized by explicit waits (ldw, st) so that each semaphore has at
    # most one DMA in flight: concurrent DMAs complete in nondeterministic
    # order, so a shared cumulative counter could release a waiter before the
    # DMA it actually depends on has landed.
    with (
        nc.Block() as block,
        nc.semaphore("ldw") as ldw,  # weight loads (serialized)
        nc.semaphore("ls0") as ls0,  # src loads, q%3 == 0
        nc.semaphore("ls1") as ls1,  # src loads, q%3 == 1
        nc.semaphore("ls2") as ls2,  # src loads, q%3 == 2
        nc.semaphore("lt0") as lt0,  # tgt loads, even quarters
        nc.semaphore("lt1") as lt1,  # tgt loads, odd quarters
        nc.semaphore("idn") as idn,  # identity built + cast
        nc.semaphore("dv") as dv,    # subs done (1 per quarter)
        nc.semaphore("ts_") as ts_,  # transpose dc-chunks done (32 per quarter)
        nc.semaphore("cp") as cp,    # diffT psum->sbuf copies (8 per quarter)
        nc.semaphore("am") as am,    # stage-A matmuls (32 per quarter)
        nc.semaphore("tc_") as tc_,  # tT copies (1 per quarter)
        nc.semaphore("bm") as bm,    # stage-B matmuls (16 per quarter)
        nc.semaphore("ad") as ad,    # adds (8 per quarter)
        nc.semaphore("st") as st,    # stores (16 per DMA, serialized)
    ):
        ls = [ls0, ls1, ls2]
        lt = [lt0, lt1]

        # Loads are split across the two HWDGE queues (sync: tgt+weights,
        # scalar: src) — a single queue streams at only ~250 GB/s; two
        # concurrent queues saturate HBM. The "(p s) d" mapping puts each
        # partition's 2 token-rows contiguous in DRAM (32 KB descriptors).
        @block.sync
        def _(sp):
            sp.dma_start(
                tgt_b[0][:],
                tgt[0:QT, :].rearrange("(p s) d -> p s d", p=128),
            ).then_inc(lt[0], 16)
            sp.dma_start(w_sb[:], w[:, :, :]).then_inc(ldw, 16)
            sp.dma_start(
                tgt_b[1][:],
                tgt[QT : 2 * QT, :].rearrange("(p s) d -> p s d", p=128),
            ).then_inc(lt[1], 16)
            sp.wait_ge(ldw, 16)
            sp.dma_start(wt_sb[:], wt[:, :]).then_inc(ldw, 16)
            for q in range(2, NQ):
                rows = slice(q * QT, (q + 1) * QT)
                # tgt buffer freed once adds(q-2) consumed it
                sp.wait_ge(ad, (q - 1) * G)
                sp.dma_start(
                    tgt_b[q % 2][:],
                    tgt[rows, :].rearrange("(p s) d -> p s d", p=128),
                ).then_inc(lt[q % 2], 16)

        @block.gpsimd
        def _(g):
            g.memset(ident_f[:], 0.0).then_inc(idn, 1)
            g.wait_ge(idn, 1)
            g.affine_select(
                out=ident_f[:],
                in_=ident_f[:],
                compare_op=mybir.AluOpType.not_equal,
                fill=1.0,
                base=0,
                pattern=[[-1, 128]],
                channel_multiplier=1,
            ).then_inc(idn, 1)
            for q in range(NQ):
                for s in range(S):
                    rows = slice(q * QT + s, (q + 1) * QT, S)
                    # adds j < 4 cover s=0, j >= 4 cover s=1
                    g.wait_ge(ad, q * G + (s + 1) * 4)
                    if q + s > 0:
                        # serialize increments of st (one store in flight)
                        g.wait_ge(st, (S * q + s) * 16)
                    g.dma_start(
                        out[rows, :],
                        src_b[q % 3][:, s, :],
                    ).then_inc(st, 16)

        @block.vector
        def _(ve):
            def sub(q):
                ve.wait_ge(ls[q % 3], (q // 3 + 1) * 16)
                ve.wait_ge(lt[q % 2], (q // 2 + 1) * 16)
                if q >= 1:
                    # single diff buffer: transposes of q-1 must be done
                    ve.wait_ge(ts_, q * DC)
                ve.tensor_sub(
                    out=diff[:], in0=src_b[q % 3][:], in1=tgt_b[q % 2][:]
                ).then_inc(dv, 1)

            def adds(q):
                for j in range(G):
                    s, c0 = j // 4, (2 * j) % NB
                    ve.wait_ge(bm, q * 16 + 2 * j + 2)
                    sl = (slice(None), s, slice(c0 * 512, (c0 + 2) * 512))
                    ve.tensor_add(
                        out=src_b[q % 3][sl], in0=p_B[j % 2][:], in1=tgt_b[q % 2][sl]
                    ).then_inc(ad, 1)

            sub(0)
            sub(1)
            adds(0)
            sub(2)
            adds(1)
            sub(3)
            adds(2)
            adds(3)

        @block.scalar
        def _(act):
            def load_src(q):
                rows = slice(q * QT, (q + 1) * QT)
                act.dma_start(
                    src_b[q % 3][:],
                    src[rows, :].rearrange("(p s) d -> p s d", p=128),
                ).then_inc(ls[q % 3], 16)

            load_src(0)
            load_src(1)
            load_src(2)
            act.wait_ge(idn, 2)
            act.copy(out=ident[:], in_=ident_f[:]).then_inc(idn, 1)
            for q in range(NQ):
                for j in range(G):
                    act.wait_ge(ts_, q * DC + 4 * j + 4)
                    if not (q == 0 and j < 2):
                        # slots freed once A-group j-2 consumed them
                        act.wait_ge(am, q * DC + 4 * j - 4)
                    act.copy(out=dT_sb[:, j % 2, :, :], in_=p_dT[j % 2][:]).then_inc(
                        cp, 1
                    )
                act.wait_ge(am, (q + 1) * DC)
                if q >= 2:
                    act.wait_ge(bm, (q - 1) * 16)
                act.copy(out=tT[:, q % 2, :], in_=p_tA[q % 2][:]).then_inc(tc_, 1)
                if q == 1:
                    # src buffer 0 freed once both stores of quarter 0 are out
                    act.wait_ge(st, 32)
                    load_src(3)

        @block.tensor
        def _(pe):
            pe.wait_ge(idn, 3)
            pe.wait_ge(ldw, 32)

            def a_group(q, g):
                for i in range(4):
                    dcc = 4 * g + i
                    if i == 0:
                        pe.wait_ge(cp, q * G + g + 1)
                        if dcc == 0 and q >= 2:
                            pe.wait_ge(tc_, q - 1)
                    pe.matmul(
                        p_tA[q % 2][:],
                        lhsT=w_sb[:, dcc, :],
                        rhs=dT_sb[:, g % 2, i, :],
                        start=(dcc == 0),
                        stop=(dcc == DC - 1),
                    ).then_inc(am, 1)

            for q in range(NQ):
                for g in range(G):
                    for i in range(4):
                        dcc = 4 * g + i
                        if dcc == 0:
                            pe.wait_ge(dv, q + 1)
                        for s in range(S):
                            t = pe.transpose(
                                p_dT[g % 2][:, i, s * 128 : (s + 1) * 128],
                                diff[:, s, dcc * 128 : (dcc + 1) * 128],
                                ident[:],
                            )
                            if s == S - 1:
                                t.then_inc(ts_, 1)
                    if g >= 1:
                        a_group(q, g - 1)
                a_group(q, G - 1)
                pe.wait_ge(tc_, q + 1)
                for k in range(16):
                    s, nb = k // NB, k % NB
                    if k % 2 == 0 and q * G + k // 2 - 1 > 0:
                        # p_B pair freed once the add two pairs back is done
                        # (crosses quarter boundaries for k < 4)
                        pe.wait_ge(ad, q * G + k // 2 - 1)
                    pe.matmul(
                        p_B[(k // 2) % 2][:, (k % 2) * 512 : (k % 2 + 1) * 512],
                        lhsT=tT[:, q % 2, s * 128 : (s + 1) * 128],
                        rhs=wt_sb[:, nb * 512 : (nb + 1) * 512],
                        start=True,
                        stop=True,
                    ).then_inc(bm, 1)

    ctx.close()
    return nc


_nc_cache = None


def _run(source, target, weight, trace=False, tmpdir=None):
    global _nc_cache
    source = np.ascontiguousarray(np.asarray(source, dtype=np.float32))
    target = np.ascontiguousarray(np.asarray(target, dtype=np.float32))
    weight = np.asarray(weight, dtype=np.float32)
    # [D, R] -> [128, DC, R] with w_r[p, o, r] = W[o*128 + p, r]
    w_bf = np.ascontiguousarray(
        weight.reshape(DC, 128, R).transpose(1, 0, 2).astype(bfloat16)
    )
    wt_bf = np.ascontiguousarray(weight.T.astype(bfloat16))
    if _nc_cache is None:
        _nc_cache = build_bass()
    nc = _nc_cache
    in_maps = []
    for c in range(N_CORES):
        rows = slice(c * TOK_PER_CORE, (c + 1) * TOK_PER_CORE)
        in_maps.append(
            {
                "source": source[rows],
                "target": target[rows],
                "weight": w_bf,
                "weight_t": wt_bf,
            }
        )
    res = run_bass_kernel_spmd(
        nc, in_maps, list(range(N_CORES)), trace=trace, tmpdir=tmpdir
    )
    full = np.concatenate([res.results[c]["out"] for c in range(N_CORES)], axis=0)
    return full, res


def kernel(source, target, weight):
    full, _ = _run(source, target, weight)
    return full
